# revision 1
# baseline (speedup 1.0000x reference)
"""Contrastive loss (SimCLR-style NT-Xent) Trainium2 kernel.

Full inputs z1, z2: [4096, 1024] f32. Output: scalar f32 loss.

Strategy (8 NeuronCores, SPMD, no collectives):
  - Host: L2-normalize rows of reps = concat(z1, z2)  [8192, 1024] (f32),
    transpose to repsT [1024, 8192], scale by 256 and cast to fp8e4m3.
  - Row-shard the 8192x8192 similarity GEMM: core c computes rows
    [c*1024, (c+1)*1024) of sim = reps @ reps.T / T against all columns,
    using fp8 DoubleRow matmuls (K=256 per instruction, ~1.4x bf16 peak).
  - The per-core program must be identical (SPMD), but the positions of the
    self-diagonal and the positive-pair diagonal inside the row block differ
    per core. Fix: feed each core B with its *columns rotated* by
    p0 = (c*1024 + 4096) mod 8192. In rotated coordinates, for every core:
      * positive-pair entries = main diagonal of columns [0, 1024)
      * self-similarity entries = diagonal of columns [4096, 5120)
    logsumexp over a row is permutation-invariant, so nothing else changes.
  - Device, per (m-tile of 128 rows, n-chunk of 2048 cols): fp8 DoubleRow
    matmuls accumulate K=1024 in 4 instructions per 512-col PSUM bank; ACT
    exp(s*x - 10) with fused per-row accumulation (accum_out) over all 4
    banks at once; on the two special chunks copy the raw f32 logit block
    out of PSUM on ACT and diag-extract on DVE (identity mul + reduce).
  - Per row i (raw scaled dot d, pos = 10*d/SCALE^2):
      T = S_full - exp(10*dself/SCALE^2 - 10) + exp(10*draw/SCALE^2 - 10)
      loss_row = (10 + ln(T)) - (10*draw/SCALE^2)
  - Host: sum the 8192 per-row values, divide by 8192.
"""

import time
from contextlib import ExitStack

import numpy as np
import ml_dtypes

import concourse.bass as bass
import concourse.tile as tile
from concourse import bacc
from concourse import mybir
from concourse import bass_utils
from concourse.masks import make_identity

B = 4096
D = 1024
S = 2 * B  # 8192 rows/cols of sim
NCORES = 8
ROWS_PER_CORE = S // NCORES  # 1024
P = 128
M_TILES = ROWS_PER_CORE // P  # 8
K_TILES = D // P  # 8
N_CHUNK = 1024  # two PSUM banks per (m, chunk) tile
N_CHUNKS = S // N_CHUNK  # 8
N_HALF = 512  # max matmul moving free dim into one PSUM bank
INV_T = 10.0  # 1 / temperature
EPS = 1e-12
FP8_SCALE = 256.0  # input scale: keeps fp8e4m3 operands in their sweet spot
SIM_SCALE = INV_T / (FP8_SCALE * FP8_SCALE)  # exp(SIM_SCALE * raw - INV_T)

_FP32 = mybir.dt.float32
_FP8 = mybir.dt.float8e4
_BF16 = mybir.dt.bfloat16
_FP8_NP = mybir.dt.np(_FP8)


def _build_bass():
    # Bacc (not raw Bass): its compile() runs generate_event_semaphores,
    # which splits multi-semaphore waits into standalone EventSemaphore
    # instructions — engine instructions can encode only one wait.
    nc = bacc.Bacc("TRN2", debug=False, num_devices=NCORES, enable_partition_id=False)
    # lhsT blocked per m-tile on the host: [m, kt, p, col] so each m-block is
    # one contiguous 128KB DMA and the PE can ramp as soon as block 0 lands.
    lhsT = nc.dram_tensor(
        "lhst", [M_TILES, K_TILES, P, P], _FP8, kind="ExternalInput"
    ).ap()
    # brot blocked per 512-column half on the host: [half, p, kt, col] so
    # each partition reads 4KB contiguous runs per half-DMA.
    brot = nc.dram_tensor(
        "brot", [S // N_HALF, P, K_TILES, N_HALF], _FP8, kind="ExternalInput"
    ).ap()
    # Raw reductions out; the tiny final combine (a few K flops) runs on the
    # host, which avoids a 1.3us ACT table switch (Ln) in the device tail.
    sums_out = nc.dram_tensor(
        "sums", [P, M_TILES * N_CHUNKS], _FP32, kind="ExternalOutput"
    ).ap()
    diag_out = nc.dram_tensor(
        "diag", [P, 2 * M_TILES], _FP32, kind="ExternalOutput"
    ).ap()

    # Pre-TileContext const region (same pattern as Bass.__init__'s
    # const_aps): values read by hot-loop instructions with no tracked
    # dependency, so they add no per-instruction sync waits. Instead of a
    # full all-engine barrier (~3us butterfly), hand off with one semaphore
    # to the only consumers (ACT reads the bias const, DVE the identity).
    bias_th = nc.alloc_sbuf_tensor("const-f32-neg10", [P, 1], _FP32)
    nc.gpsimd.memset(bias_th.ap(), -INV_T)
    nc.const_aps.aps[(_FP32, -INV_T)] = bias_th.ap()
    ident_th = nc.alloc_sbuf_tensor("identity-f32", [P, P], _FP32)
    nc.gpsimd.memset(ident_th.ap(), 0.0)
    ident_inst = nc.gpsimd.affine_select(
        out=ident_th.ap(),
        in_=ident_th.ap(),
        compare_op=mybir.AluOpType.not_equal,
        fill=1.0,
        base=0,
        pattern=[[-1, P]],
        channel_multiplier=1,
    )
    const_sem = nc.alloc_semaphore("const-ready")
    ident_inst.then_inc(const_sem, 1)
    nc.vector.wait_ge(const_sem, 1)
    nc.scalar.wait_ge(const_sem, 1)

    with tile.TileContext(nc) as tc:
        _body(tc, lhsT, brot, sums_out, diag_out, ident_th.ap())
    nc.compile()
    return nc


def _body(tc, lhsT, brot, sums_out, diag_out, ident):
    nc = tc.nc
    AF = mybir.ActivationFunctionType

    # DRAM views with partition dim first: [p, kt, ...]
    a_view = lhsT.rearrange("m k p c -> p m k c")  # [128, 8, 8, 128]

    ctx = ExitStack()
    singles = ctx.enter_context(tc.tile_pool(name="singles", bufs=1))
    bpool = ctx.enter_context(tc.tile_pool(name="bchunks", bufs=3))
    # 4 tiles x 2 banks: deep PSUM pipeline so matmuls never wait on the
    # ACT exp/read-accumulator chain of the tile being recycled.
    pspool = ctx.enter_context(tc.tile_pool(name="psum", bufs=4, space="PSUM"))
    # Exp elementwise outputs are write-only garbage (the fused accum_out is
    # what we keep); bf16 halves their SBUF footprint.
    epool = ctx.enter_context(tc.tile_pool(name="exps", bufs=8))
    # Single-use slots for the 16 diagonal extractions.
    scratch = ctx.enter_context(tc.tile_pool(name="scratch", bufs=16))

    # Resident stationary operand: all local rows, transposed. SBUF layout
    # [p, kt, m*128+col]; m-block 0 is loaded before the first b chunk, the
    # rest right after it (the PE consumes m-blocks at ~2us each, so they
    # arrive well ahead).
    a_t = singles.tile([P, K_TILES, ROWS_PER_CORE], _FP8)

    def load_a_block(m):
        nc.sync.dma_start(
            out=a_t[:, :, m * P : (m + 1) * P], in_=a_view[:, m, :, :]
        )

    load_a_block(0)

    # Per-row partial sums: column m*N_CHUNKS + nch. Disjoint-column writes
    # carry no WAW dependencies between the exps.
    sums = singles.tile([P, M_TILES * N_CHUNKS], _FP32)
    # Raw (pre-exp, scaled) diagonal values: cols [0:8] positive, [8:16] self.
    diag = singles.tile([P, 2 * M_TILES], _FP32)

    for nch in range(N_CHUNKS):
        b_t = bpool.tile([P, K_TILES, N_CHUNK], _FP8)
        # Two half-loads (columns) so matmuls on the first PSUM bank can
        # start while the second half is still arriving.
        nc.sync.dma_start(out=b_t[:, :, 0:N_HALF], in_=brot[2 * nch])
        nc.sync.dma_start(out=b_t[:, :, N_HALF:N_CHUNK], in_=brot[2 * nch + 1])
        if nch == 0:
            for mb in range(1, M_TILES):
                load_a_block(mb)
        for m in range(M_TILES):
            ps = pspool.tile([P, N_CHUNK], _FP32)
            col = m * N_CHUNKS + nch
            for half in range(N_CHUNK // N_HALF):
                hs = slice(half * N_HALF, (half + 1) * N_HALF)
                for kt in range(0, K_TILES, 2):
                    nc.tensor.matmul(
                        ps[:, hs],
                        a_t[:, kt : kt + 2, m * P : (m + 1) * P],
                        b_t[:, kt : kt + 2, hs],
                        start=(kt == 0),
                        stop=(kt == K_TILES - 2),
                        perf_mode=mybir.MatmulPerfMode.DoubleRow,
                    )
            # exp over both PSUM banks at once; fused per-row accumulation.
            # All PE-group RAW waits share one semaphore.
            e_t = epool.tile([P, N_CHUNK], _BF16)
            nc.scalar.activation(
                out=e_t,
                in_=ps,
                func=AF.Exp,
                bias=-INV_T,
                scale=SIM_SCALE,
                accum_out=sums[:, col : col + 1],
            )
            # Diagonal extraction on the two special chunks. In rotated
            # coords, m-tile m's positive diagonal lives at columns
            # [m*128, (m+1)*128) -> chunk 0, offset 128*m; the self
            # diagonal at columns [4096 + m*128, ...) -> chunk 4.
            dcol = None
            if nch == 0:
                dcol = m
            elif nch == 4:
                dcol = M_TILES + m
            if dcol is not None:
                # DVE extracts the raw f32 diagonal straight from PSUM
                # (identity mul + reduce); Bacc's generate_event_semaphores
                # legalizes the resulting extra WAR wait on the recycling
                # matmul, and this keeps the ACT engine (the pipeline's
                # second-busiest) free of copy work.
                off = m * P
                diag_t = scratch.tile([P, P], _FP32)
                nc.vector.tensor_mul(diag_t, ps[:, off : off + P], ident)
                nc.vector.reduce_sum(
                    diag[:, dcol : dcol + 1], diag_t, axis=mybir.AxisListType.X
                )

        if nch == 4:
            # Both diagonals are complete; ship them while chunks 5-7 run.
            nc.sync.dma_start(out=diag_out, in_=diag)

    nc.sync.dma_start(out=sums_out, in_=sums)
    ctx.close()


_NC_CACHE = {}


def _get_nc():
    if "nc" not in _NC_CACHE:
        _NC_CACHE["nc"] = _build_bass()
    return _NC_CACHE["nc"]


def _make_in_maps(z1, z2):
    z1 = np.asarray(z1, dtype=np.float32)
    z2 = np.asarray(z2, dtype=np.float32)
    z = np.concatenate([z1, z2], axis=0)  # [8192, 1024]
    nrm = np.sqrt(np.sum(z * z, axis=1, keepdims=True, dtype=np.float32))
    n = z / np.maximum(nrm, EPS)
    repsT = np.ascontiguousarray(n.T * FP8_SCALE).astype(_FP8_NP)  # [1024, 8192]
    in_maps = []
    for c in range(NCORES):
        p0 = ((c * ROWS_PER_CORE) + B) % S
        rolled = np.concatenate([repsT[:, p0:], repsT[:, :p0]], axis=1)
        lhsT_c = repsT[:, c * ROWS_PER_CORE : (c + 1) * ROWS_PER_CORE]
        # Block per m-tile: [m, kt, p, col]
        lhsT_blk = np.ascontiguousarray(
            lhsT_c.reshape(K_TILES, P, M_TILES, P).transpose(2, 0, 1, 3)
        )
        # Block per 512-col half: [half, p, kt, col]
        b_blk = np.ascontiguousarray(
            rolled.reshape(K_TILES, P, S // N_HALF, N_HALF).transpose(2, 1, 0, 3)
        )
        in_maps.append({"lhst": lhsT_blk, "brot": b_blk})
    return in_maps


def _combine(results):
    # Per row i: T = S_full - e_self + e_pos; loss_row = ln(T) - (pos - 10)
    # with pos - 10 = SIM_SCALE*draw - 10. A few K flops; done in f64.
    total = 0.0
    for r in results:
        stot = r["sums"].astype(np.float64).reshape(P, M_TILES, N_CHUNKS).sum(axis=2)
        diag = r["diag"].astype(np.float64)
        draw, dself = diag[:, :M_TILES], diag[:, M_TILES:]
        e_pos = np.exp(SIM_SCALE * draw - INV_T)
        e_self = np.exp(SIM_SCALE * dself - INV_T)
        loss_rows = np.log(stot - e_self + e_pos) - (SIM_SCALE * draw - INV_T)
        total += float(loss_rows.sum())
    return np.array(total / S, dtype=np.float32)


def run_traced(z1, z2, **spmd_kwargs):
    """Run on HW with profiling; returns (loss, BassKernelResults)."""
    nc = _get_nc()
    in_maps = _make_in_maps(z1, z2)
    res = bass_utils.run_bass_kernel_spmd(
        nc, in_maps, core_ids=list(range(NCORES)), trace=True, **spmd_kwargs
    )
    return _combine(res.results), res


def kernel(z1, z2):
    nc = _get_nc()
    in_maps = _make_in_maps(z1, z2)
    last_err = None
    for _attempt in range(3):
        try:
            res = bass_utils.run_bass_kernel_spmd(
                nc, in_maps, core_ids=list(range(NCORES))
            )
            return _combine(res.results)
        except Exception as e:  # transient device wedge: retry
            last_err = e
            time.sleep(2.0)
    raise last_err



# revision 4
# speedup vs baseline: 1.6473x; 1.6473x over previous
"""Contrastive loss (SimCLR-style NT-Xent) Trainium2 kernel — symmetric GEMM.

Full inputs z1, z2: [4096, 1024] f32. Output: scalar f32 loss.

sim = reps @ reps.T is symmetric, so only ~half the 8192x8192 GEMM needs
computing. Core c owns rows [c*1024, (c+1)*1024) and computes (fp8 DoubleRow,
K=256/instr):
  - self block (c, c):   m-tile m computes cols [128m, 1024)   (upper tri)
  - blocks (c, c+d), d=1..3: full 1024 cols
  - far block (c, c+4):  m-tile m computes cols [128m, 1024)   (upper tri)
That is 4.125 of 8 block-columns -> ~2x fewer matmul cycles than the full
row-sharded GEMM. The transposed halves are recovered on the host from
per-column sums of exp (partition-partial csum tiles, reduced on host):
  - block (c, c-d) row sums come from core (c-d)'s column sums of (c-d, c)
  - the lower triangles of self/far come from the same core's/partner's
    column sums; the doubly-counted 128x128 diagonal subtiles are removed
    with per-row diag-subtile sums (dsub) computed by DVE from the exp tile.
Positives live on the far block's diagonal subtiles, self-sims on the self
block's; both raw diagonals are extracted by DVE (fused mul+reduce) for the
host's exact logaddexp correction.

Per (m, chunk): PE accumulates K=1024 into a 2-bank PSUM tile (4 fp8
DoubleRow matmuls per <=512-col span); ACT does exp(s*x - 10) with fused
per-row accumulation; DVE folds the exp tile into the running column sums
and extracts diagonals. Device outputs raw partials; the tiny final combine
(a few M flops) runs on the host in f64.

SPMD: all 8 cores run the identical program; each core's input map carries
its own row block (a) and its blocks c+1..c+4 (b), so no rotation and no
collectives are needed.
"""

import time
from contextlib import ExitStack

import numpy as np
import ml_dtypes

import concourse.bass as bass
import concourse.tile as tile
from concourse import bacc
from concourse import mybir
from concourse import bass_utils

B = 4096
D = 1024
S = 2 * B  # 8192 rows/cols of sim
NCORES = 8
RPC = S // NCORES  # 1024 rows per core
P = 128
M_TILES = RPC // P  # 8
K_TILES = D // P  # 8
INV_T = 10.0  # 1 / temperature
EPS = 1e-12
FP8_SCALE = 256.0  # input scale: keeps fp8e4m3 operands in their sweet spot
SIM_SCALE = INV_T / (FP8_SCALE * FP8_SCALE)  # exp(SIM_SCALE * raw - INV_T)

_FP32 = mybir.dt.float32
_FP8 = mybir.dt.float8e4
_BF16 = mybir.dt.bfloat16
_FP8_NP = mybir.dt.np(_FP8)

# out tile column layout ([128, 72] f32)
SL_SELF = 0      # 8 cols: row sums, self chunk (per m)
SL_D = 8         # 24 cols: row sums, d=1..3 chunks (8*(d-1)+m)
SL_DIAG_S = 32   # 8 cols: raw self diag
SL_DSUB_S = 40   # 8 cols: exp rowsum over self diag subtile
SL_FAR = 48      # 8 cols: row sums, far chunk
SL_DIAG_F = 56   # 8 cols: raw positive diag
SL_DSUB_F = 64   # 8 cols: exp rowsum over far diag subtile
N_OUT = 72
# csum_out column layout ([128, 5120] f32): partition-partial column sums
CS_SELF = 0      # [0, 1024): self block
CS_D = 1024      # [1024, 4096): blocks c+1..c+3
CS_FAR = 4096    # [4096, 5120): far block


def _build_bass():
    # Bacc (not raw Bass): its compile() runs generate_event_semaphores,
    # which splits multi-semaphore waits into standalone EventSemaphore
    # instructions — engine instructions can encode only one wait.
    nc = bacc.Bacc("TRN2", debug=False, num_devices=NCORES, enable_partition_id=False)
    # a blocked per m-tile: [mb, p, kt, c]; per-partition runs of 1KB. Loaded
    # m-descending so the self chunk (processed m=7..0) can start after the
    # first 128KB strip instead of the full 1MB.
    a_dram = nc.dram_tensor(
        "a", [M_TILES, P, K_TILES, P], _FP8, kind="ExternalInput"
    ).ap()
    # b: blocks c+1..c+4 as 512-col halves: [h, p, kt, 512]; 4KB runs.
    b_dram = nc.dram_tensor(
        "b", [8, P, K_TILES, 512], _FP8, kind="ExternalInput"
    ).ap()
    out_dram = nc.dram_tensor("out", [P, N_OUT], _FP32, kind="ExternalOutput").ap()
    csum_dram = nc.dram_tensor(
        "csum", [P, 5 * RPC], _FP32, kind="ExternalOutput"
    ).ap()

    # Pre-TileContext const region (same pattern as Bass.__init__'s
    # const_aps): values read by hot-loop instructions with no tracked
    # dependency, so they add no per-instruction sync waits. Hand off with
    # one semaphore to the only consumers (ACT bias const, DVE identity).
    bias_th = nc.alloc_sbuf_tensor("const-f32-neg10", [P, 1], _FP32)
    nc.gpsimd.memset(bias_th.ap(), -INV_T)
    nc.const_aps.aps[(_FP32, -INV_T)] = bias_th.ap()
    ident_th = nc.alloc_sbuf_tensor("identity-f32", [P, P], _FP32)
    ms_inst = nc.gpsimd.memset(ident_th.ap(), 0.0)
    # Same-engine in-order on HW; the explicit edge keeps CoreSim's race
    # detector satisfied.
    ms_sem = nc.alloc_semaphore("ident-zeroed")
    ms_inst.then_inc(ms_sem, 1)
    nc.gpsimd.wait_ge(ms_sem, 1)
    ident_inst = nc.gpsimd.affine_select(
        out=ident_th.ap(),
        in_=ident_th.ap(),
        compare_op=mybir.AluOpType.not_equal,
        fill=1.0,
        base=0,
        pattern=[[-1, P]],
        channel_multiplier=1,
    )
    const_sem = nc.alloc_semaphore("const-ready")
    ident_inst.then_inc(const_sem, 1)
    nc.vector.wait_ge(const_sem, 1)
    nc.scalar.wait_ge(const_sem, 1)

    with tile.TileContext(nc) as tc:
        _body(tc, a_dram, b_dram, out_dram, csum_dram, ident_th.ap())
    nc.compile()
    return nc


def _spans(w):
    """Bank-aligned <=512-col spans covering [0, w)."""
    return [(s, min(s + 512, w)) for s in range(0, w, 512)]


def _body(tc, a_dram, b_dram, out_dram, csum_dram, ident):
    nc = tc.nc
    AF = mybir.ActivationFunctionType
    ALU = mybir.AluOpType

    ctx = ExitStack()
    singles = ctx.enter_context(tc.tile_pool(name="singles", bufs=1))
    # 4 tiles x 2 banks: deep PSUM pipeline so matmuls never wait on the
    # ACT exp/read-accumulator chain of the tile being recycled.
    pspool = ctx.enter_context(tc.tile_pool(name="psum", bufs=4, space="PSUM"))
    # Exp tiles (bf16): consumed by DVE column-sum accumulation.
    epool = ctx.enter_context(tc.tile_pool(name="exps", bufs=6))
    # Write-only garbage for the fused diag extractions.
    scratch = ctx.enter_context(tc.tile_pool(name="scratch", bufs=4))

    # Resident operands: own rows (a_t, also the self chunk's columns) and
    # blocks c+1..c+4 (b_t). All loaded up front; PE consumes ~57us of
    # matmul, the 5MB streams in well ahead.
    a_t = singles.tile([P, K_TILES, RPC], _FP8)
    b_t = singles.tile([P, K_TILES, 4 * RPC], _FP8)

    out_t = singles.tile([P, N_OUT], _FP32)
    csum_s = singles.tile([P, RPC], _FP32)
    csum_d = singles.tile([P, 3 * RPC], _FP32)
    csum_f = singles.tile([P, RPC], _FP32)

    # a strips m=7..0 (self phase runs m descending), then b halves in
    # consumption order.
    for mb in range(M_TILES - 1, -1, -1):
        nc.sync.dma_start(out=a_t[:, :, mb * P : (mb + 1) * P], in_=a_dram[mb])
    for h in range(8):
        nc.sync.dma_start(out=b_t[:, :, h * 512 : (h + 1) * 512], in_=b_dram[h])

    def mm_tile(ps, m, w, mov, mov_off):
        """ps[:, 0:w] = a-rows m-tile x mov columns [mov_off, mov_off+w)."""
        for s0, s1 in _spans(w):
            for kt in range(0, K_TILES, 2):
                nc.tensor.matmul(
                    ps[:, s0:s1],
                    a_t[:, kt : kt + 2, m * P : (m + 1) * P],
                    mov[:, kt : kt + 2, mov_off + s0 : mov_off + s1],
                    start=(kt == 0),
                    stop=(kt == K_TILES - 2),
                    perf_mode=mybir.MatmulPerfMode.DoubleRow,
                )

    def act_exp(ps, w, slot, e_t):
        nc.scalar.activation(
            out=e_t[:, :w],
            in_=ps[:, :w],
            func=AF.Exp,
            bias=-INV_T,
            scale=SIM_SCALE,
            accum_out=out_t[:, slot : slot + 1],
        )

    def diag_extract(ps, e_t, dcol, scol):
        # Raw diagonal from PSUM (identity mul + row reduce) and the exp
        # rowsum over the 128x128 diag subtile (from the bf16 exp tile).
        diag_t = scratch.tile([P, P], _FP32)
        nc.vector.tensor_mul(diag_t, ps[:, 0:P], ident)
        nc.vector.reduce_sum(
            out_t[:, dcol : dcol + 1], diag_t, axis=mybir.AxisListType.X
        )
        nc.vector.reduce_sum(
            out_t[:, scol : scol + 1], e_t[:, 0:P], axis=mybir.AxisListType.X
        )

    # --- self chunk (block c), m descending: triangle cols [128m, 1024) ---
    for m in range(M_TILES - 1, -1, -1):
        w = RPC - m * P
        ps = pspool.tile([P, 1024], _FP32)
        mm_tile(ps, m, w, a_t, m * P)
        e_t = epool.tile([P, 1024], _BF16)
        act_exp(ps, w, SL_SELF + m, e_t)
        diag_extract(ps, e_t, SL_DIAG_S + m, SL_DSUB_S + m)
        # Column-sum accumulate: new 128-col strip is a copy, the rest adds.
        nc.vector.tensor_copy(csum_s[:, m * P : (m + 1) * P], e_t[:, 0:P])
        if w > P:
            nc.vector.tensor_add(
                csum_s[:, (m + 1) * P : RPC],
                csum_s[:, (m + 1) * P : RPC],
                e_t[:, P:w],
            )
    nc.sync.dma_start(out=csum_dram[:, CS_SELF : CS_SELF + RPC], in_=csum_s)

    # --- d = 1..3 chunks (blocks c+d), full 1024 cols ---
    for d in (1, 2, 3):
        boff = (d - 1) * RPC
        for m in range(M_TILES):
            ps = pspool.tile([P, 1024], _FP32)
            mm_tile(ps, m, RPC, b_t, boff)
            e_t = epool.tile([P, 1024], _BF16)
            act_exp(ps, RPC, SL_D + (d - 1) * M_TILES + m, e_t)
            if m == 0:
                nc.vector.tensor_copy(csum_d[:, boff : boff + RPC], e_t)
            else:
                nc.vector.tensor_add(
                    csum_d[:, boff : boff + RPC],
                    csum_d[:, boff : boff + RPC],
                    e_t,
                )
        nc.sync.dma_start(
            out=csum_dram[:, CS_D + boff : CS_D + boff + RPC],
            in_=csum_d[:, boff : boff + RPC],
        )

    # Everything except the far chunk's outputs is final; ship it while the
    # far chunk computes.
    nc.sync.dma_start(out=out_dram[:, 0:SL_FAR], in_=out_t[:, 0:SL_FAR])

    # --- far chunk (block c+4), m ascending: triangle cols [128m, 1024) ---
    foff = 3 * RPC
    for m in range(M_TILES):
        w = RPC - m * P
        ps = pspool.tile([P, 1024], _FP32)
        mm_tile(ps, m, w, b_t, foff + m * P)
        e_t = epool.tile([P, 1024], _BF16)
        act_exp(ps, w, SL_FAR + m, e_t)
        diag_extract(ps, e_t, SL_DIAG_F + m, SL_DSUB_F + m)
        if m == 0:
            nc.vector.tensor_copy(csum_f, e_t)
        else:
            nc.vector.tensor_add(
                csum_f[:, m * P : RPC], csum_f[:, m * P : RPC], e_t[:, 0:w]
            )
        # Strip [128m, 128(m+1)) takes no further adds: ship progressively.
        nc.sync.dma_start(
            out=csum_dram[:, CS_FAR + m * P : CS_FAR + (m + 1) * P],
            in_=csum_f[:, m * P : (m + 1) * P],
        )

    nc.sync.dma_start(out=out_dram[:, SL_FAR:N_OUT], in_=out_t[:, SL_FAR:N_OUT])
    ctx.close()


_NC_CACHE = {}


def _get_nc():
    if "nc" not in _NC_CACHE:
        _NC_CACHE["nc"] = _build_bass()
    return _NC_CACHE["nc"]


def _make_in_maps(z1, z2):
    z1 = np.asarray(z1, dtype=np.float32)
    z2 = np.asarray(z2, dtype=np.float32)
    z = np.concatenate([z1, z2], axis=0)  # [8192, 1024]
    nrm = np.sqrt(np.sum(z * z, axis=1, keepdims=True, dtype=np.float32))
    n = z / np.maximum(nrm, EPS)
    repsT = np.ascontiguousarray(n.T * FP8_SCALE).astype(_FP8_NP)  # [1024, 8192]
    in_maps = []
    for c in range(NCORES):
        own = repsT[:, c * RPC : (c + 1) * RPC]  # [1024(K), 1024]
        # [mb, p, kt, col]
        a_blk = np.ascontiguousarray(
            own.reshape(K_TILES, P, M_TILES, P).transpose(2, 1, 0, 3)
        )
        # blocks c+1..c+4 as halves: [h, p, kt, 512]
        blocks = []
        for d in (1, 2, 3, 4):
            bc = (c + d) % NCORES
            cols = repsT[:, bc * RPC : (bc + 1) * RPC]
            blocks.append(cols.reshape(K_TILES, P, 2, 512).transpose(2, 1, 0, 3))
        b_blk = np.ascontiguousarray(np.concatenate(blocks, axis=0))
        in_maps.append({"a": a_blk, "b": b_blk})
    return in_maps


def _combine(results):
    # Assemble per-row negative-mass totals from row sums + column sums,
    # fix the doubly-counted diag subtiles, apply the pos/self diagonal
    # corrections, reduce. A few M flops in f64.
    outs = [r["out"].astype(np.float64) for r in results]
    csums = [r["csum"].astype(np.float64) for r in results]
    # colsum[c][x]: full column sums (over the computing core's 128
    # partitions) for each of the 5 chunk column ranges.
    colsum = [cs.sum(axis=0) for cs in csums]  # [5120] each

    def rowvals(o, base):  # out cols [base, base+8) -> per-row vector [1024]
        return o[:, base : base + M_TILES].T.reshape(-1)  # r = 128m + p

    total = 0.0
    for c in range(NCORES):
        o = outs[c]
        main_self = rowvals(o, SL_SELF)
        main_d = sum(rowvals(o, SL_D + (d - 1) * M_TILES) for d in (1, 2, 3))
        diag_s = rowvals(o, SL_DIAG_S)
        dsub_s = rowvals(o, SL_DSUB_S)
        main_far = rowvals(o, SL_FAR)
        diag_f = rowvals(o, SL_DIAG_F)
        dsub_f = rowvals(o, SL_DSUB_F)
        col_other = np.zeros(RPC)
        for d in (1, 2, 3):
            cs = colsum[(c - d) % NCORES]
            col_other = col_other + cs[CS_D + (d - 1) * RPC : CS_D + d * RPC]
        col_far = colsum[(c + 4) % NCORES][CS_FAR : CS_FAR + RPC]
        col_self = colsum[c][CS_SELF : CS_SELF + RPC]
        S_i = (
            main_self + col_self - dsub_s
            + main_d + col_other
            + main_far + col_far - dsub_f
        )
        e_pos = np.exp(SIM_SCALE * diag_f - INV_T)
        e_self = np.exp(SIM_SCALE * diag_s - INV_T)
        loss_rows = np.log(S_i + e_pos - e_self) - (SIM_SCALE * diag_f - INV_T)
        total += float(loss_rows.sum())
    return np.array(total / S, dtype=np.float32)


def run_traced(z1, z2, **spmd_kwargs):
    """Run on HW with profiling; returns (loss, BassKernelResults)."""
    nc = _get_nc()
    in_maps = _make_in_maps(z1, z2)
    res = bass_utils.run_bass_kernel_spmd(
        nc, in_maps, core_ids=list(range(NCORES)), trace=True, **spmd_kwargs
    )
    return _combine(res.results), res


def kernel(z1, z2):
    nc = _get_nc()
    in_maps = _make_in_maps(z1, z2)
    last_err = None
    for _attempt in range(3):
        try:
            res = bass_utils.run_bass_kernel_spmd(
                nc, in_maps, core_ids=list(range(NCORES))
            )
            return _combine(res.results)
        except Exception as e:  # transient device wedge: retry
            last_err = e
            time.sleep(2.0)
    raise last_err
